# revision 48
# baseline (speedup 1.0000x reference)
"""Trainium2 Bass kernel for nn_AttentionBlock (B=8, C=512, HW=32x32).

Sharding: pure data-parallel over batch — each of the 8 NeuronCores computes
one batch item end-to-end; no collectives. All weights are replicated.

Restructured for PE warmth + engine balance (~434us baseline -> ~252us):
  - Shared PSUM pool with two 2-slot rings (tags "sc" rotating short-lived
    tiles / "pv" long-lived accumulators = 8 banks total) so the tensor
    stream stays dense across phase borders.
  - Softmax denominators come free from the PV matmul: V is stored
    interleaved with an all-ones 64-column block ([V_h | 1]), so PV psum rows
    64:127 hold the per-query denominator replicated across 64 partitions.
    The old ones-matmul sm chains for the 8 std heads are gone.
  - PV + denominator are drained out of PSUM immediately (copies first, on
    vector), freeing the banks; the 6.5us vector reciprocal and the comb
    multiply run decoupled from the tensor stream.
  - Attention is software-pipelined at MM granularity: scores of pair i+1
    (and the ms-branch-0 scores as "pair 4"), plus the multi-scale
    projections, are interleaved into the PV stream of pair i, so the PE
    never idles >3.4us (HAM stays at K=8/8 instead of oscillating).
  - The conv gate is interleaved with q/k projection chunks (its silu-gated
    stages would stall the in-order tensor queue).
  - ms branches run as slot-disciplined passes: br1 scores + br0 denominator,
    br0 PV, br1 denominator (reciprocal overlaps) + br1 PV; the std half of
    all four FFN first-layer accumulations is pre-issued under that drain.
  - GroupNorm phases are chunked: x is DMA'd per 128-chan block, bn_stats per
    chunk; applies split across ScalarE (activation Identity w/ per-partition
    scale+bias AP) and VectorE; x is re-DMA'd for the residual so its SBUF
    can be recycled during attention.
fp8/DoubleRow was tried and REVERTED: random-sign contractions do not average
fp8 quantization error down (measured rel_err 0.049 > 0.02 gate).
"""

import contextlib
import numpy as np

import concourse.bass as bass
import concourse.tile as tile
from concourse import mybir
from concourse.vector_clock import ScopedClock

F32 = mybir.dt.float32
BF16 = mybir.dt.bfloat16
F8 = mybir.dt.float8e4
AF = mybir.ActivationFunctionType
ALU = mybir.AluOpType

B, C, HW = 8, 512, 32
S = HW * HW            # 1024 tokens
NH = 8                 # heads
HD = C // NH           # 64
NG = 32                # groups
GS = C // NG           # 16 channels per group
EPS = 1e-6
NB = C // 128          # 4 channel blocks
NK = S // 128          # 8 token blocks
CH = C // 2            # 256 multi-branch dim

# ---------------------------------------------------------------------------
# Workaround: this container's walrus rejects more than one sync-wait per
# sequencer ctrl instruction; Tile's kernel-tail drain accumulates one wait
# per live semaphore.  Spread the waits over single-wait NOPs instead.
# ---------------------------------------------------------------------------


def _patched_drain_and_barrier(self, tick_clock, wait_clock):
    nc = self.nc
    drain_inst = nc.sync.drain()
    wait_clock.add_sem_waits(
        drain_inst.ins, ScopedClock({None: tick_clock.global_clock})
    )
    si = drain_inst.ins.sync_info
    if si is not None and len(si.on_wait) > 0:
        waits = list(si.on_wait)
        drain_inst.ins.sync_info = mybir.SyncInfo(
            on_wait=[], on_update=list(si.on_update)
        )
        for w in waits:
            nop = nc.sync.nop(nofuse=True)
            nop.ins.sync_info = mybir.SyncInfo(on_wait=[w], on_update=[])

    nc.all_engine_barrier()
    assert self.sems is not None
    popped = nc._tile_sem_poison_stack.pop()
    assert popped is self._sem_poison
    nc.clear_and_free_semaphores(list(self.sems.allocated().values()))
    nc.all_engine_barrier()


tile.TileContext._drain_and_barrier = _patched_drain_and_barrier


def _split_multi_waits(nc, max_waits=1):
    """Hoist excess semaphore waits onto same-engine NOPs placed just before
    the owning instruction (engines execute their stream in order, so waiting
    on a preceding NOP is equivalent)."""
    for f in nc.m.functions:
        for bb in f.blocks:
            insts = list(bb.instructions)
            if not any(
                i.sync_info is not None and len(i.sync_info.on_wait) > max_waits
                for i in insts
            ):
                continue
            out = []
            for inst in insts:
                si = inst.sync_info
                if si is not None and len(si.on_wait) > max_waits:
                    waits = list(si.on_wait)
                    extra, keep = waits[:-max_waits], waits[-max_waits:]
                    for j in range(0, len(extra), max_waits):
                        nop = mybir.InstNoOp(
                            name=f"I-wsplit-{nc.next_id()}", ins=[], outs=[],
                            engine=inst.engine,
                        )
                        nop.sync_info = mybir.SyncInfo(
                            on_wait=extra[j:j + max_waits], on_update=[]
                        )
                        out.append(nop)
                    inst.sync_info = mybir.SyncInfo(
                        on_wait=keep, on_update=list(si.on_update)
                    )
                out.append(inst)
            bb.instructions = out


# ---------------------------------------------------------------------------
# Graph builder
# ---------------------------------------------------------------------------
def build(nc: bass.Bass):
    d = {}
    P = nc.declare_dram_parameter

    d["x"] = P("x", [NB, 128, S], F32, isOutput=False)
    d["xh"] = P("xh", [NB, 128, S], BF16, isOutput=False)
    d["pos"] = P("pos", [NB, 128, S], BF16, isOutput=False)
    for n in ("pre_g", "pre_b", "norm_g", "norm_b", "post_g", "post_b"):
        d[n] = P(n, [128, NB], F32, isOutput=False)
    d["wqt"] = P("wqt", [NB, 128, C], BF16, isOutput=False)
    d["wkt"] = P("wkt", [NB, 128, C], BF16, isOutput=False)
    d["wvt"] = P("wvt", [NB, 128, C], BF16, isOutput=False)
    d["wmt"] = P("wmt", [6, NB, 128, CH], BF16, isOutput=False)  # q0 k0 v0 q1 k1 v1
    d["w1ff"] = P("w1ff", [2 * NB, 128, C], BF16, isOutput=False)
    d["w2ff"] = P("w2ff", [NB, 128, C], BF16, isOutput=False)
    d["wout"] = P("wout", [NB, 128, C], BF16, isOutput=False)
    d["ffb1"] = P("ffb1", [128, NB], F32, isOutput=False)
    d["outbe"] = P("outbe", [128, NB], F32, isOutput=False)  # wout@ffb2 + outb
    d["cw1"] = P("cw1", [NB, 128, 64], BF16, isOutput=False)
    d["cw2"] = P("cw2", [9, 64, 64], BF16, isOutput=False)
    d["cw3"] = P("cw3", [64, 1], BF16, isOutput=False)
    d["cb1"] = P("cb1", [64, 1], F32, isOutput=False)
    d["cb2"] = P("cb2", [64, 1], F32, isOutput=False)
    d["cb3"] = P("cb3", [1, 1], F32, isOutput=False)
    d["gsel"] = P("gsel", [128, 8], F32, isOutput=False)   # [p, j] = 1/16 if p//16==j
    d["gbct"] = P("gbct", [8, 128], F32, isOutput=False)   # [j, p] = 1 if p//16==j
    d["out"] = P("out", [NB, 128, S], BF16, isOutput=True)

    with tile.TileContext(nc, num_cores=8) as tc:
        _body(nc, tc, d)
    _split_multi_waits(nc)
    return nc


def _gn_coeffs(nc, psp, spool, mv, gamma, beta, gsel_t, gbct_t, eps_t, nm):
    """GroupNorm coefficients from per-(row, block) mean/var stats
    mv [128, NB, 2]: returns per-channel f32 (scale, shift) tiles such that
    y = x*scale + shift."""
    # rq [128, 8]: cols 0..3 = per-row mean (per blk), 4..7 = per-row E[x^2]
    rq = spool.tile([128, 8], F32, tag="gn_rq", name=f"rq{nm}")
    nc.vector.tensor_copy(out=rq[:, 0:NB], in_=mv[:, :, 0])
    nc.vector.scalar_tensor_tensor(
        out=rq[:, NB:2 * NB], in0=mv[:, :, 0], scalar=1.0, in1=mv[:, :, 0],
        op0=ALU.mult, op1=ALU.mult,
    )
    nc.vector.tensor_add(out=rq[:, NB:2 * NB], in0=rq[:, NB:2 * NB], in1=mv[:, :, 1])
    # group-combine (mean over each group's 16 partitions): [8, 8]
    gst_ps = psp.tile([8, 8], F32, tag="sc", name=f"gstps{nm}")
    nc.tensor.matmul(out=gst_ps, lhsT=gsel_t, rhs=rq, start=True, stop=True)
    gst = spool.tile([8, 8], F32, tag="gn_gst", name=f"gst{nm}")
    nc.vector.tensor_copy(out=gst, in_=gst_ps)
    grs = spool.tile([8, NB], F32, tag="gn_grs", name=f"grs{nm}")
    nc.vector.scalar_tensor_tensor(
        out=grs, in0=gst[:, 0:NB], scalar=-1.0, in1=gst[:, 0:NB],
        op0=ALU.mult, op1=ALU.mult,
    )
    nc.vector.tensor_add(out=grs, in0=grs, in1=gst[:, NB:2 * NB])
    nc.scalar.activation(out=grs, in_=grs, func=AF.Sqrt, bias=eps_t[0:8], scale=1.0)
    nc.vector.reciprocal(out=grs, in_=grs)
    # broadcast group values back to channel partitions: [128, 8]
    gmr = spool.tile([8, 8], F32, tag="gn_gmr", name=f"gmr{nm}")
    nc.vector.tensor_copy(out=gmr[:, 0:NB], in_=gst[:, 0:NB])
    nc.vector.tensor_copy(out=gmr[:, NB:2 * NB], in_=grs)
    pcs_ps = psp.tile([128, 8], F32, tag="sc", name=f"pcsps{nm}")
    nc.tensor.matmul(out=pcs_ps, lhsT=gbct_t, rhs=gmr, start=True, stop=True)
    scale = spool.tile([128, NB], F32, tag="gn_scale", name=f"sc{nm}")
    shift = spool.tile([128, NB], F32, tag="gn_shift", name=f"sh{nm}")
    nc.vector.tensor_mul(out=scale, in0=pcs_ps[:, NB:2 * NB], in1=gamma)
    nc.vector.tensor_mul(out=shift, in0=pcs_ps[:, 0:NB], in1=scale)
    nc.vector.tensor_sub(out=shift, in0=beta, in1=shift)
    return scale, shift


def _body(nc, tc, d):
    ctx = contextlib.ExitStack()
    with ctx:
        consts = ctx.enter_context(tc.tile_pool(name="consts", bufs=1))
        gnp = ctx.enter_context(tc.tile_pool(name="gnp", bufs=2))
        psp = ctx.enter_context(tc.tile_pool(name="psp", bufs=2, space="PSUM"))

        # persistent activation pools (created first: stack-order allocation
        # requires the phase-scoped pools below to sit on top)
        shp = ctx.enter_context(tc.tile_pool(name="shp", bufs=1))
        sh_t = shp.tile([128, NB, S], BF16, tag="sh")
        msh = shp.tile([128, NB, S], BF16, tag="msh")
        qkvp = ctx.enter_context(tc.tile_pool(name="qkvp", bufs=1))
        q_t = qkvp.tile([128, NB, S], BF16, tag="q")
        k_t = qkvp.tile([128, NB, S], BF16, tag="k")
        # V interleaved with an all-ones block: [.., h, 0:64]=V_h, [.., 64:128]=1
        vt2 = qkvp.tile([128, NK, NH, 128], BF16, tag="vt")
        nc.any.memset(vt2.rearrange("p k h d -> p (k h) d")[:, :, 64:128], 1.0)
        qm_t = qkvp.tile([128, 2, 2, S], BF16, tag="qm")   # [p, br, mchunk, S]
        km_t = qkvp.tile([128, 2, 2, S], BF16, tag="km")
        vmt_t = qkvp.tile([128, 2, NK, CH], BF16, tag="vmt")
        combp = ctx.enter_context(tc.tile_pool(name="combp", bufs=1))
        comb = combp.tile([128, 2 * NB, S], BF16, tag="comb")

        # ---- input DMAs (x first so GN1 stats can start ASAP) --------------
        ph12 = contextlib.ExitStack()  # pools freed once x/hs/pos are dead
        xp = ph12.enter_context(tc.tile_pool(name="xp", bufs=1))
        # bf16 copy of x for GN1 stats + apply (halves the startup-critical
        # DMA; full-precision x is re-fetched later for the residual)
        x_t = xp.tile([128, NB, S], BF16, tag="x")
        for b in range(NB):
            nc.sync.dma_start(out=x_t[:, b, :], in_=d["xh"].ap()[b])

        posp = ph12.enter_context(tc.tile_pool(name="posp", bufs=1))
        pos_t = posp.tile([128, NB, S], BF16, tag="pos")
        for b in range(NB):
            nc.sync.dma_start(out=pos_t[:, b, :], in_=d["pos"].ap()[b])

        def const_tile(name, shape, dtype):
            t = consts.tile(shape, dtype, tag=name, name=name)
            nc.sync.dma_start(out=t, in_=d[name].ap())
            return t

        def const_blocks(name, nb, w, dtype=BF16):
            t = consts.tile([128, nb, w], dtype, tag=name, name=name)
            nc.sync.dma_start(out=t, in_=d[name].ap().rearrange("b p w -> p b w"))
            return t

        # small / early-needed constants
        gsel_t = const_tile("gsel", [128, 8], F32)
        gbct_t = const_tile("gbct", [8, 128], F32)
        gn_par = {}
        for n in ("pre_g", "pre_b", "norm_g", "norm_b", "post_g", "post_b"):
            gn_par[n] = const_tile(n, [128, NB], F32)
        cw1 = const_blocks("cw1", NB, 64)
        cw2 = consts.tile([64, 9, 64], BF16, tag="cw2", name="cw2")
        nc.sync.dma_start(out=cw2, in_=d["cw2"].ap().rearrange("t p w -> p t w"))
        cw3 = const_tile("cw3", [64, 1], BF16)
        cb1 = const_tile("cb1", [64, 1], F32)
        cb2 = const_tile("cb2", [64, 1], F32)
        cb3 = const_tile("cb3", [1, 1], F32)
        # std projection weights (needed right after GN2)
        wqt = const_blocks("wqt", NB, C)
        wkt = const_blocks("wkt", NB, C)
        wvt = const_blocks("wvt", NB, C)
        # late weights
        wmt = consts.tile([128, 6, NB, CH], BF16, tag="wmt", name="wmt")
        nc.sync.dma_start(out=wmt, in_=d["wmt"].ap().rearrange("i b p w -> p i b w"))
        w1ff = const_blocks("w1ff", 2 * NB, C)
        w2ff = const_blocks("w2ff", NB, C)
        wout = const_blocks("wout", NB, C)
        ffb1 = const_tile("ffb1", [128, NB], F32)
        outbe = const_tile("outbe", [128, NB], F32)

        eps_t = consts.tile([128, 1], F32, tag="eps", name="eps")
        nc.vector.memset(eps_t, EPS)
        ones_mat = consts.tile([128, 128], BF16, tag="ones_mat", name="ones_mat")
        nc.vector.memset(ones_mat, 1.0)
        ones_row = consts.tile([1, 128], BF16, tag="ones_row", name="ones_row")
        nc.vector.memset(ones_row, 1.0)
        zw64 = consts.tile([128, 64], BF16, tag="zw64", name="zw64")
        nc.vector.memset(zw64, 0.0)

        # ---- phase 1: GN1 + pos -> hs ; GN2 -> sh --------------------------
        ph1 = contextlib.ExitStack()
        hsp = ph12.enter_context(tc.tile_pool(name="hsp", bufs=1))
        hs = hsp.tile([128, NB, S], BF16, tag="hs")
        with ph1:
            tmpp = ph1.enter_context(tc.tile_pool(name="tmpp", bufs=2))
            mv1 = gnp.tile([128, NB, 2], F32, tag="gn_mv", name="mv1")
            for b in range(NB):
                st = gnp.tile([128, 2, 6], F32, tag="gn_st", name=f"st1_{b}")
                for h in range(2):
                    nc.vector.bn_stats(out=st[:, h, :],
                                       in_=x_t[:, b, 512 * h:512 * (h + 1)])
                nc.vector.bn_aggr(out=mv1[:, b, :], in_=st)
            sc1, sh1 = _gn_coeffs(nc, psp, gnp, mv1, gn_par["pre_g"],
                                  gn_par["pre_b"], gsel_t, gbct_t, eps_t, "1")
            mv2 = gnp.tile([128, NB, 2], F32, tag="gn_mv", name="mv2")
            for b in range(NB):
                tmp = tmpp.tile([128, S], BF16, tag="hs_tmp", name=f"hst{b}")
                nc.scalar.activation(out=tmp, in_=x_t[:, b, :], func=AF.Identity,
                                     bias=sh1[:, b:b + 1], scale=sc1[:, b:b + 1])
                nc.vector.tensor_add(out=hs[:, b, :], in0=tmp, in1=pos_t[:, b, :])
                st = gnp.tile([128, 2, 6], F32, tag="gn_st", name=f"st2_{b}")
                for h in range(2):
                    nc.vector.bn_stats(out=st[:, h, :],
                                       in_=hs[:, b, 512 * h:512 * (h + 1)])
                nc.vector.bn_aggr(out=mv2[:, b, :], in_=st)
            sc2, sh2 = _gn_coeffs(nc, psp, gnp, mv2, gn_par["norm_g"],
                                  gn_par["norm_b"], gsel_t, gbct_t, eps_t, "2")
            for b in range(NB):
                nc.vector.tensor_scalar(
                    out=sh_t[:, b, :], in0=hs[:, b, :],
                    scalar1=sc2[:, b:b + 1], scalar2=sh2[:, b:b + 1],
                    op0=ALU.mult, op1=ALU.add,
                )

        # ---- phase 2+3: conv spatial gate interleaved with q/k projections -
        # (conv's silu-gated stages would stall the in-order tensor queue, so
        # independent projection chunks are slotted between them)
        def qk_chunk(w_t, dst, m, nm):
            pst = psp.tile([128, S], F32, tag="sc", name=f"proj{nm}{m}")
            for b in range(NB):
                for n in range(2):
                    nc.tensor.matmul(
                        out=pst[:, 512 * n:512 * (n + 1)],
                        lhsT=w_t[:, b, 128 * m:128 * (m + 1)],
                        rhs=sh_t[:, b, 512 * n:512 * (n + 1)],
                        start=(b == 0), stop=(b == NB - 1),
                    )
            nc.any.tensor_copy(out=dst[:, m, :], in_=pst)

        ph2 = contextlib.ExitStack()
        with ph2:
            convp = ph2.enter_context(tc.tile_pool(name="convp", bufs=1))
            PW = 34
            # conv1 1x1 -> silu -> a1 (bf16, zero-padded 34x34)
            c1ps = psp.tile([64, S], F32, tag="pv", name="c1ps")
            for b in range(NB):
                for n in range(2):
                    nc.tensor.matmul(
                        out=c1ps[:, 512 * n:512 * (n + 1)],
                        lhsT=cw1[:, b, :], rhs=hs[:, b, 512 * n:512 * (n + 1)],
                        start=(b == 0), stop=(b == NB - 1),
                    )
            a1b = convp.tile([64, PW * PW], BF16, tag="a1b")
            nc.vector.memset(a1b, 0.0)
            a1bv = a1b.rearrange("p (h w) -> p h w", h=PW)
            c1v = c1ps.rearrange("p (h w) -> p h w", h=HW)
            # cb1 rewritten through a GN2-scale-dependent no-op: orders this
            # silu after GN2's Sqrt in the scalar queue (avoids reloading the
            # sqrt table set between the two) without touching the tensor
            # stream
            cb1b = convp.tile([64, 1], F32, tag="cb1b")
            nc.vector.tensor_scalar(out=cb1b, in0=sc2[0:64, 0:1],
                                    scalar1=0.0, scalar2=None, op0=ALU.mult)
            nc.vector.tensor_add(out=cb1b, in0=cb1b, in1=cb1)
            nc.scalar.activation(out=a1bv[:, 1:33, 1:33], in_=c1v, func=AF.Silu,
                                 bias=cb1b, scale=1.0)
            qk_chunk(wqt, q_t, 0, "q")
            # conv2 3x3 -> silu -> a2 (bf16, padded)
            a2b = convp.tile([64, PW * PW], BF16, tag="a2b")
            nc.vector.memset(a2b, 0.0)
            a2bv = a2b.rearrange("p (h w) -> p h w", h=PW)
            for half in range(2):
                c2ps = psp.tile([64, 512], F32, tag="pv", name=f"c2ps{half}")
                for tap in range(9):
                    ky, kx = tap // 3, tap % 3
                    rv = a1bv[:, 16 * half + ky:16 * half + ky + 16, kx:kx + 32]
                    nc.tensor.matmul(
                        out=c2ps, lhsT=cw2[:, tap, :], rhs=rv,
                        start=(tap == 0), stop=(tap == 8),
                    )
                c2v = c2ps.rearrange("p (h w) -> p h w", h=16)
                nc.scalar.activation(
                    out=a2bv[:, 1 + 16 * half:17 + 16 * half, 1:33],
                    in_=c2v, func=AF.Silu, bias=cb2, scale=1.0,
                )
                qk_chunk(wkt if half else wqt, k_t if half else q_t, 1, "ck")
            # conv3 1x1 -> sigmoid -> sw [1, S]
            c3ps = psp.tile([1, S], F32, tag="pv", name="c3ps")
            for n in range(2):
                rv = a2bv[:, 1 + 16 * n:17 + 16 * n, 1:33]
                nc.tensor.matmul(out=c3ps[:, 512 * n:512 * (n + 1)], lhsT=cw3,
                                 rhs=rv, start=True, stop=True)
            sw = convp.tile([1, S], BF16, tag="sw")
            nc.scalar.activation(out=sw, in_=c3ps, func=AF.Sigmoid, bias=cb3,
                                 scale=1.0)
            qk_chunk(wqt, q_t, 2, "q")
            # broadcast over channels; msh = hs * sigmoid(...)
            swb_ps = psp.tile([128, S], F32, tag="pv", name="swbps")
            for n in range(2):
                nc.tensor.matmul(out=swb_ps[:, 512 * n:512 * (n + 1)],
                                 lhsT=ones_row, rhs=sw[:, 512 * n:512 * (n + 1)],
                                 start=True, stop=True)
            swb = convp.tile([128, S], BF16, tag="swb")
            nc.vector.tensor_copy(out=swb, in_=swb_ps)
            for b in range(NB):
                nc.any.tensor_mul(out=msh[:, b, :], in0=hs[:, b, :], in1=swb)
        ph12.close()  # hs / pos dead; free their SBUF for attention pools

        # ---- remaining projections ----------------------------------------
        inv_sqrt_hd = float(1.0 / np.sqrt(HD))
        inv_sqrt_ch = float(1.0 / np.sqrt(CH))
        qk_chunk(wqt, q_t, 3, "q")
        for m in (0, 2, 3):
            qk_chunk(wkt, k_t, m, "k")


        ph4 = contextlib.ExitStack()
        with ph4:
            ptp = ph4.enter_context(tc.tile_pool(name="ptp", bufs=3))
            rrp = ph4.enter_context(tc.tile_pool(name="rrp", bufs=3))
            pvsp = ph4.enter_context(tc.tile_pool(name="pvsp", bufs=2))

            def sc_exp(pts, ch, kc, hh):
                scp = psp.tile([128, S], F32, tag="sc", name=f"sc{ch}_{kc}_{hh}")
                for n in range(2):
                    nc.tensor.matmul(
                        out=scp[:, 512 * n:512 * (n + 1)],
                        lhsT=k_t[64 * hh:64 * (hh + 1), ch, 128 * kc:128 * (kc + 1)],
                        rhs=q_t[64 * hh:64 * (hh + 1), ch, 512 * n:512 * (n + 1)],
                        start=True, stop=True,
                        tile_position=(64 * hh, 0),
                    )
                nc.scalar.activation(out=pts[hh][:, kc, :], in_=scp,
                                     func=AF.Exp, scale=inv_sqrt_hd)

            def vt2_chunk(m):
                # V^T chunk [128 tokens, C], scattered per-head into vt2
                pst = psp.tile([128, C], F32, tag="sc", name=f"projv{m}")
                for b in range(NB):
                    nc.tensor.matmul(
                        out=pst, lhsT=sh_t[:, b, 128 * m:128 * (m + 1)],
                        rhs=wvt[:, b, :], start=(b == 0), stop=(b == NB - 1),
                    )
                nc.any.tensor_copy(
                    out=vt2[:, m, :, 0:64],
                    in_=pst.rearrange("p (h v) -> p h v", h=NH),
                )

            def ms_proj_group(i):
                # i=0: qm/km br0; i=1: qm/km br1; i=2: vmt br0; i=3: vmt br1
                if i < 2:
                    br = i
                    for (wi, src, dst) in ((0, msh, qm_t), (1, sh_t, km_t)):
                        for m in range(2):
                            pst = psp.tile([128, S], F32, tag="sc",
                                           name=f"pm{br}{wi}{m}")
                            for b in range(NB):
                                for n in range(2):
                                    nc.tensor.matmul(
                                        out=pst[:, 512 * n:512 * (n + 1)],
                                        lhsT=wmt[:, 3 * br + wi, b,
                                                 128 * m:128 * (m + 1)],
                                        rhs=src[:, b, 512 * n:512 * (n + 1)],
                                        start=(b == 0), stop=(b == NB - 1),
                                    )
                            nc.scalar.copy(out=dst[:, br, m, :], in_=pst)
                else:
                    br = i - 2
                    for m in range(NK):
                        pst = psp.tile([128, CH], F32, tag="sc",
                                       name=f"pvm{br}{m}")
                        for b in range(NB):
                            nc.tensor.matmul(
                                out=pst,
                                lhsT=sh_t[:, b, 128 * m:128 * (m + 1)],
                                rhs=wmt[:, 3 * br + 2, b, :],
                                start=(b == 0), stop=(b == NB - 1),
                            )
                        nc.scalar.copy(out=vmt_t[:, br, m, :], in_=pst)

            def ms_sc_exp(ptm, br, kc):
                scp = psp.tile([128, S], F32, tag="sc", name=f"msc{br}_{kc}")
                for n in range(2):
                    for b2 in range(2):
                        nc.tensor.matmul(
                            out=scp[:, 512 * n:512 * (n + 1)],
                            lhsT=km_t[:, br, b2, 128 * kc:128 * (kc + 1)],
                            rhs=qm_t[:, br, b2, 512 * n:512 * (n + 1)],
                            start=(b2 == 0), stop=(b2 == 1),
                        )
                nc.scalar.activation(out=ptm[:, kc, :], in_=scp,
                                     func=AF.Exp, scale=inv_sqrt_ch)

            # pair 0 scores interleaved with V-chunk projections
            pts_cur = [ptp.tile([128, NK, S], BF16, tag="pt", name=f"pt0_{i}")
                       for i in range(2)]
            for kc in range(NK):
                sc_exp(pts_cur, 0, kc, 0)
                sc_exp(pts_cur, 0, kc, 1)
                vt2_chunk(kc)

            # pipelined pairs: PV of pair i + scores of pair i+1 (the multi-
            # scale branch-0 scores play "pair 4" inside pair 3's loop).
            ptm0 = None
            for i in range(NB):
                nxt = None
                if i + 1 < NB:
                    nxt = [ptp.tile([128, NK, S], BF16, tag="pt",
                                    name=f"pt{i + 1}_{j}") for j in range(2)]
                else:
                    ptm0 = ptp.tile([128, NK, S], BF16, tag="pt", name="ptm0")
                pvp = [None, None]
                for hh in range(2):
                    pvp[hh] = psp.tile([128, S], F32, tag="pv",
                                       name=f"pv{i}_{hh}")
                for kc in range(NK):
                    if nxt is not None:
                        sc_exp(nxt, i + 1, kc, 0)
                        sc_exp(nxt, i + 1, kc, 1)
                    else:
                        ms_sc_exp(ptm0, 0, kc)
                    for hh in range(2):
                        h = 2 * i + hh
                        for n in range(2):
                            nc.tensor.matmul(
                                out=pvp[hh][:, 512 * n:512 * (n + 1)],
                                lhsT=vt2[:, kc, h, :],
                                rhs=pts_cur[hh][:, kc, 512 * n:512 * (n + 1)],
                                start=(kc == 0), stop=(kc == NK - 1),
                            )
                # drain PV+denominator out of PSUM fast (frees the banks for
                # the next pair) BEFORE any reciprocal hits the vector queue
                dens, pvsbs = [], []
                for hh in range(2):
                    den = pvsp.tile([64, S], F32, tag="den", name=f"den{i}{hh}")
                    pvsb = pvsp.tile([64, S], BF16, tag="pvsb",
                                     name=f"pvsb{i}{hh}")
                    nc.vector.tensor_copy(out=den, in_=pvp[hh][64:128, :])
                    nc.vector.tensor_copy(out=pvsb, in_=pvp[hh][0:64, :])
                    dens.append(den)
                    pvsbs.append(pvsb)
                for hh in range(2):
                    rr = rrp.tile([64, S], F32, tag="rr", name=f"rr{i}{hh}")
                    nc.vector.reciprocal(out=rr, in_=dens[hh])
                    nc.any.tensor_mul(
                        out=comb[64 * hh:64 * (hh + 1), i, :],
                        in0=pvsbs[hh], in1=rr)
                ms_proj_group(i)
                pts_cur = nxt

            # pass A: branch-1 scores interleaved with branch-0 denominator
            ptm1 = ptp.tile([128, NK, S], BF16, tag="pt", name="ptm1")
            smp0 = psp.tile([128, S], F32, tag="pv", name="smp0")
            for kc in range(NK):
                ms_sc_exp(ptm1, 1, kc)
                for n in range(2):
                    nc.tensor.matmul(
                        out=smp0[:, 512 * n:512 * (n + 1)], lhsT=ones_mat,
                        rhs=ptm0[:, kc, 512 * n:512 * (n + 1)],
                        start=(kc == 0), stop=(kc == NK - 1),
                    )
            densm0 = pvsp.tile([128, S], F32, tag="den", name="densm0")
            nc.vector.tensor_copy(out=densm0, in_=smp0)
            rrm0 = rrp.tile([128, S], F32, tag="rr", name="rrm0")
            nc.vector.reciprocal(out=rrm0, in_=densm0)

            # pass B: branch-0 PV
            pvm0 = [psp.tile([128, S], F32, tag="pv", name=f"pvm0_{m}")
                    for m in range(2)]
            for kc in range(NK):
                for m in range(2):
                    for n in range(2):
                        nc.tensor.matmul(
                            out=pvm0[m][:, 512 * n:512 * (n + 1)],
                            lhsT=vmt_t[:, 0, kc, 128 * m:128 * (m + 1)],
                            rhs=ptm0[:, kc, 512 * n:512 * (n + 1)],
                            start=(kc == 0), stop=(kc == NK - 1),
                        )
            for m in range(2):
                pvsbm = pvsp.tile([128, S], BF16, tag="pvsb",
                                  name=f"pvsbm0{m}")
                nc.vector.tensor_copy(out=pvsbm, in_=pvm0[m])
                nc.any.tensor_mul(out=comb[:, NB + m, :], in0=pvsbm, in1=rrm0)

            # pass C: branch-1 denominator FIRST (its reciprocal then overlaps
            # the branch-1 PV matmuls), then PV
            smp1 = psp.tile([128, S], F32, tag="sc", name="smp1")
            for kc in range(NK):
                for n in range(2):
                    nc.tensor.matmul(
                        out=smp1[:, 512 * n:512 * (n + 1)], lhsT=ones_mat,
                        rhs=ptm1[:, kc, 512 * n:512 * (n + 1)],
                        start=(kc == 0), stop=(kc == NK - 1),
                    )
            densm1 = pvsp.tile([128, S], F32, tag="den", name="densm1")
            nc.vector.tensor_copy(out=densm1, in_=smp1)
            rrm1 = rrp.tile([128, S], F32, tag="rr", name="rrm1")
            nc.vector.reciprocal(out=rrm1, in_=densm1)
            pvm1 = [psp.tile([128, S], F32, tag="pv", name=f"pvm1_{m}")
                    for m in range(2)]
            for kc in range(NK):
                for m in range(2):
                    for n in range(2):
                        nc.tensor.matmul(
                            out=pvm1[m][:, 512 * n:512 * (n + 1)],
                            lhsT=vmt_t[:, 1, kc, 128 * m:128 * (m + 1)],
                            rhs=ptm1[:, kc, 512 * n:512 * (n + 1)],
                            start=(kc == 0), stop=(kc == NK - 1),
                        )
            for m in range(2):
                pvsbm = pvsp.tile([128, S], BF16, tag="pvsb",
                                  name=f"pvsbm1{m}")
                nc.vector.tensor_copy(out=pvsbm, in_=pvm1[m])
                nc.any.tensor_mul(out=comb[:, NB + 2 + m, :], in0=pvsbm,
                                  in1=rrm1)
            # std half of the first FFN accumulations fills the PE while the
            # branch-1 normalize drains on vector
            ff1_pre = [psp.tile([128, S], F32,
                                tag=("sc" if m < 2 else "pv"),
                                name=f"ff1_{m}")
                       for m in range(4)]
            for m in range(4):
                for b in range(NB):
                    for n in range(2):
                        nc.tensor.matmul(
                            out=ff1_pre[m][:, 512 * n:512 * (n + 1)],
                            lhsT=w1ff[:, b, 128 * m:128 * (m + 1)],
                            rhs=comb[:, b, 512 * n:512 * (n + 1)],
                            start=(b == 0), stop=False,
                        )

        # ---- phase 5: FFN + out projection ---------------------------------
        ffp = ctx.enter_context(tc.tile_pool(name="ffp", bufs=1))
        # re-fetch x (bf16) for the residual (x_t's SBUF was recycled after
        # phase 1); bf16 keeps the residual adds in the fast 2x DVE mode
        x2 = ffp.tile([128, NB, S], BF16, tag="x2")
        for b in range(NB):
            nc.sync.dma_start(out=x2[:, b, :], in_=d["xh"].ap()[b])
        act1 = ffp.tile([128, NB, S], BF16, tag="act1")
        for m in range(NB):
            pst = ff1_pre[m]   # std half already accumulated
            for b in range(NB, 2 * NB):
                for n in range(2):
                    nc.tensor.matmul(
                        out=pst[:, 512 * n:512 * (n + 1)],
                        lhsT=w1ff[:, b, 128 * m:128 * (m + 1)],
                        rhs=comb[:, b, 512 * n:512 * (n + 1)],
                        start=False, stop=(b == 2 * NB - 1),
                    )
            nc.scalar.activation(out=act1[:, m, :], in_=pst, func=AF.Silu,
                                 bias=ffb1[:, m:m + 1], scale=1.0)
        f2o = ffp.tile([128, NB, S], BF16, tag="f2o")
        for m in range(NB):
            pst = psp.tile([128, S], F32, tag=("pv" if m % 2 else "sc"), name=f"ff2_{m}")
            for b in range(NB):
                for n in range(2):
                    nc.tensor.matmul(
                        out=pst[:, 512 * n:512 * (n + 1)],
                        lhsT=w2ff[:, b, 128 * m:128 * (m + 1)],
                        rhs=act1[:, b, 512 * n:512 * (n + 1)],
                        start=(b == 0), stop=(b == NB - 1),
                    )
            nc.any.tensor_copy(out=f2o[:, m, :], in_=pst)
        final = ffp.tile([128, NB, S], F32, tag="final")
        mv3 = gnp.tile([128, NB, 2], F32, tag="gn_mv", name="mv3")
        for m in range(NB):
            pst = psp.tile([128, S], F32, tag=("pv" if m % 2 else "sc"), name=f"ffo_{m}")
            for b in range(NB):
                for n in range(2):
                    nc.tensor.matmul(
                        out=pst[:, 512 * n:512 * (n + 1)],
                        lhsT=wout[:, b, 128 * m:128 * (m + 1)],
                        rhs=f2o[:, b, 512 * n:512 * (n + 1)],
                        start=(b == 0), stop=(b == NB - 1),
                    )
            nc.scalar.activation(out=final[:, m, :], in_=pst, func=AF.Identity,
                                 bias=outbe[:, m:m + 1], scale=1.0)
            # GN3 stats straight off the PSUM, in parallel with the bias-apply
            # above; variance is shift-invariant, the mean gets the outbe
            # shift added afterwards
            st = gnp.tile([128, 2, 6], F32, tag="gn_st", name=f"st3_{m}")
            for h in range(2):
                nc.vector.bn_stats(out=st[:, h, :],
                                   in_=pst[:, 512 * h:512 * (h + 1)])
            nc.vector.bn_aggr(out=mv3[:, m, :], in_=st)
            nc.vector.tensor_add(out=mv3[:, m, 0:1], in0=mv3[:, m, 0:1],
                                 in1=outbe[:, m:m + 1])

        # ---- phase 6: GN3 + residual -> out --------------------------------
        sc3, sh3 = _gn_coeffs(nc, psp, gnp, mv3, gn_par["post_g"],
                              gn_par["post_b"], gsel_t, gbct_t, eps_t, "3")
        outt = ffp.tile([128, NB, S], BF16, tag="outt")
        for b in range(NB):
            # alternate engines so the four block-tails pipeline two-wide
            if b % 2 == 0:
                nc.scalar.activation(out=outt[:, b, :], in_=final[:, b, :],
                                     func=AF.Identity, bias=sh3[:, b:b + 1],
                                     scale=sc3[:, b:b + 1])
                nc.vector.tensor_add(out=outt[:, b, :], in0=outt[:, b, :],
                                     in1=x2[:, b, :])
            else:
                nc.vector.tensor_scalar(
                    out=outt[:, b, :], in0=final[:, b, :],
                    scalar1=sc3[:, b:b + 1], scalar2=sh3[:, b:b + 1],
                    op0=ALU.mult, op1=ALU.add,
                )
                nc.gpsimd.tensor_add(out=outt[:, b, :], in0=outt[:, b, :],
                                     in1=x2[:, b, :])
            nc.sync.dma_start(out=d["out"].ap()[b], in_=outt[:, b, :])


# ---------------------------------------------------------------------------
# Host wrapper
# ---------------------------------------------------------------------------
def _prep_inputs(inputs):
    import ml_dtypes
    bf = ml_dtypes.bfloat16
    f8 = ml_dtypes.float8_e4m3
    f32 = np.float32

    def t(a):  # [out, in] -> [in, out]
        return np.ascontiguousarray(np.asarray(a, f32).T)

    def blocks(a, nb):  # [in, out] -> [nb, 128, out]
        return np.ascontiguousarray(np.asarray(a).reshape(nb, 128, -1))

    def chan(a):  # [512] -> [128, 4], channel c = 128*blk + p
        return np.ascontiguousarray(np.asarray(a, f32).reshape(NB, 128).T)

    common = {}
    common["pos"] = np.asarray(inputs["pos_emb"], f32).reshape(C, S).reshape(
        NB, 128, S).astype(bf)
    for n in ("pre_g", "pre_b", "norm_g", "norm_b", "post_g", "post_b"):
        common[n] = chan(inputs[n])
    common["wqt"] = blocks(t(inputs["wq"]), NB).astype(bf)
    common["wkt"] = blocks(t(inputs["wk"]), NB).astype(bf)
    common["wvt"] = blocks(t(inputs["wv"]), NB).astype(bf)
    wm = [t(inputs[k]) for k in ("wq0", "wk0", "wv0", "wq1", "wk1", "wv1")]
    common["wmt"] = np.stack([blocks(w, NB) for w in wm]).astype(bf)
    common["w1ff"] = blocks(t(inputs["ff_w1"]), 2 * NB).astype(bf)
    common["w2ff"] = blocks(t(inputs["ff_w2"]), NB).astype(bf)
    common["wout"] = blocks(t(inputs["out_w"]), NB).astype(bf)
    common["ffb1"] = chan(inputs["ff_b1"])
    out_b_eff = (np.asarray(inputs["out_w"], f32) @ np.asarray(inputs["ff_b2"], f32)
                 + np.asarray(inputs["out_b"], f32))
    common["outbe"] = chan(out_b_eff)
    common["cw1"] = blocks(t(np.asarray(inputs["sa_w1"], f32)[:, :, 0, 0]), NB)\
        .astype(bf)
    w2 = np.asarray(inputs["sa_w2"], f32)  # (64, 64, 3, 3) OIHW
    common["cw2"] = np.stack(
        [np.ascontiguousarray(w2[:, :, ky, kx].T)
         for ky in range(3) for kx in range(3)]).astype(bf)
    common["cw3"] = t(np.asarray(inputs["sa_w3"], f32)[:, :, 0, 0]).astype(bf)
    common["cb1"] = np.asarray(inputs["sa_b1"], f32).reshape(64, 1)
    common["cb2"] = np.asarray(inputs["sa_b2"], f32).reshape(64, 1)
    common["cb3"] = np.asarray(inputs["sa_b3"], f32).reshape(1, 1)
    gsel = np.zeros((128, 8), f32)
    for p in range(128):
        gsel[p, p // GS] = 1.0 / GS
    common["gsel"] = gsel
    gbct = np.zeros((8, 128), f32)
    for p in range(128):
        gbct[p // GS, p] = 1.0
    common["gbct"] = gbct

    xs = np.asarray(inputs["hidden_states"], f32).reshape(B, C, S)
    in_maps = []
    for b in range(B):
        m = dict(common)
        m["x"] = np.ascontiguousarray(xs[b].reshape(NB, 128, S))
        m["xh"] = m["x"].astype(bf)
        in_maps.append(m)
    return in_maps


def kernel(**inputs):
    from concourse.bass_utils import run_bass_kernel_spmd

    nc = bass.Bass("TRN2", debug=False, num_devices=8)
    build(nc)
    in_maps = _prep_inputs(inputs)
    res = run_bass_kernel_spmd(nc, in_maps, core_ids=list(range(8)))
    out = np.stack([np.asarray(res.results[i]["out"], np.float32)
                    for i in range(B)])
    return out.reshape(B, C, HW, HW)


if __name__ == "__main__":
    import reference
    inp = {k: np.asarray(v) for k, v in reference.setup_inputs().items()}
    exp = np.asarray(reference.reference(**inp))
    got = kernel(**inp)
    rel = float(np.linalg.norm(got - exp) / np.linalg.norm(exp))
    print("max abs err:", float(np.abs(got - exp).max()), "rel:", rel)


# revision 49
# speedup vs baseline: 1.2457x; 1.2457x over previous
"""Trainium2 Bass kernel for nn_AttentionBlock (B=8, C=512, HW=32x32).

Sharding: pure data-parallel over batch — each of the 8 NeuronCores computes
one batch item end-to-end; no collectives. All weights are replicated.

Restructured for PE warmth + engine balance (~434us baseline -> ~252us):
  - Shared PSUM pool with two 2-slot rings (tags "sc" rotating short-lived
    tiles / "pv" long-lived accumulators = 8 banks total) so the tensor
    stream stays dense across phase borders.
  - Softmax denominators come free from the PV matmul: V is stored
    interleaved with an all-ones 64-column block ([V_h | 1]), so PV psum rows
    64:127 hold the per-query denominator replicated across 64 partitions.
    The old ones-matmul sm chains for the 8 std heads are gone.
  - PV + denominator are drained out of PSUM immediately (copies first, on
    vector), freeing the banks; the 6.5us vector reciprocal and the comb
    multiply run decoupled from the tensor stream.
  - Attention is software-pipelined at MM granularity: scores of pair i+1
    (and the ms-branch-0 scores as "pair 4"), plus the multi-scale
    projections, are interleaved into the PV stream of pair i, so the PE
    never idles >3.4us (HAM stays at K=8/8 instead of oscillating).
  - The conv gate is interleaved with q/k projection chunks (its silu-gated
    stages would stall the in-order tensor queue).
  - ms branches run as slot-disciplined passes: br1 scores + br0 denominator,
    br0 PV, br1 denominator (reciprocal overlaps) + br1 PV; the std half of
    all four FFN first-layer accumulations is pre-issued under that drain.
  - GroupNorm phases are chunked: x is DMA'd per 128-chan block, bn_stats per
    chunk; applies split across ScalarE (activation Identity w/ per-partition
    scale+bias AP) and VectorE; x is re-DMA'd for the residual so its SBUF
    can be recycled during attention.
fp8/DoubleRow was tried and REVERTED: random-sign contractions do not average
fp8 quantization error down (measured rel_err 0.049 > 0.02 gate).
"""

import contextlib
import numpy as np

import concourse.bass as bass
import concourse.tile as tile
from concourse import mybir
from concourse.vector_clock import ScopedClock

F32 = mybir.dt.float32
BF16 = mybir.dt.bfloat16
F8 = mybir.dt.float8e4
AF = mybir.ActivationFunctionType
ALU = mybir.AluOpType

B, C, HW = 8, 512, 32
S = HW * HW            # 1024 tokens
NH = 8                 # heads
HD = C // NH           # 64
NG = 32                # groups
GS = C // NG           # 16 channels per group
EPS = 1e-6
NB = C // 128          # 4 channel blocks
NK = S // 128          # 8 token blocks
CH = C // 2            # 256 multi-branch dim

# ---------------------------------------------------------------------------
# Workaround: this container's walrus rejects more than one sync-wait per
# sequencer ctrl instruction; Tile's kernel-tail drain accumulates one wait
# per live semaphore.  Spread the waits over single-wait NOPs instead.
# ---------------------------------------------------------------------------


def _patched_drain_and_barrier(self, tick_clock, wait_clock):
    nc = self.nc
    drain_inst = nc.sync.drain()
    wait_clock.add_sem_waits(
        drain_inst.ins, ScopedClock({None: tick_clock.global_clock})
    )
    si = drain_inst.ins.sync_info
    if si is not None and len(si.on_wait) > 0:
        waits = list(si.on_wait)
        drain_inst.ins.sync_info = mybir.SyncInfo(
            on_wait=[], on_update=list(si.on_update)
        )
        for w in waits:
            nop = nc.sync.nop(nofuse=True)
            nop.ins.sync_info = mybir.SyncInfo(on_wait=[w], on_update=[])

    nc.all_engine_barrier()
    assert self.sems is not None
    popped = nc._tile_sem_poison_stack.pop()
    assert popped is self._sem_poison
    nc.clear_and_free_semaphores(list(self.sems.allocated().values()))
    nc.all_engine_barrier()


tile.TileContext._drain_and_barrier = _patched_drain_and_barrier


def _split_multi_waits(nc, max_waits=1):
    """Hoist excess semaphore waits onto same-engine NOPs placed just before
    the owning instruction (engines execute their stream in order, so waiting
    on a preceding NOP is equivalent)."""
    for f in nc.m.functions:
        for bb in f.blocks:
            insts = list(bb.instructions)
            if not any(
                i.sync_info is not None and len(i.sync_info.on_wait) > max_waits
                for i in insts
            ):
                continue
            out = []
            for inst in insts:
                si = inst.sync_info
                if si is not None and len(si.on_wait) > max_waits:
                    waits = list(si.on_wait)
                    extra, keep = waits[:-max_waits], waits[-max_waits:]
                    for j in range(0, len(extra), max_waits):
                        nop = mybir.InstNoOp(
                            name=f"I-wsplit-{nc.next_id()}", ins=[], outs=[],
                            engine=inst.engine,
                        )
                        nop.sync_info = mybir.SyncInfo(
                            on_wait=extra[j:j + max_waits], on_update=[]
                        )
                        out.append(nop)
                    inst.sync_info = mybir.SyncInfo(
                        on_wait=keep, on_update=list(si.on_update)
                    )
                out.append(inst)
            bb.instructions = out


# ---------------------------------------------------------------------------
# Graph builder
# ---------------------------------------------------------------------------
def build(nc: bass.Bass):
    d = {}
    P = nc.declare_dram_parameter

    d["x"] = P("x", [NB, 128, S], F32, isOutput=False)
    d["xh"] = P("xh", [NB, 128, S], BF16, isOutput=False)
    d["pos"] = P("pos", [NB, 128, S], BF16, isOutput=False)
    for n in ("pre_g", "pre_b", "norm_g", "norm_b", "post_g", "post_b"):
        d[n] = P(n, [128, NB], F32, isOutput=False)
    d["wqt"] = P("wqt", [NB, 128, C], BF16, isOutput=False)
    d["wkt"] = P("wkt", [NB, 128, C], BF16, isOutput=False)
    d["wvt"] = P("wvt", [NB, 128, C], BF16, isOutput=False)
    d["wmt"] = P("wmt", [6, NB, 128, CH], BF16, isOutput=False)  # q0 k0 v0 q1 k1 v1
    d["w1ff"] = P("w1ff", [2 * NB, 128, C], BF16, isOutput=False)
    d["w2ff"] = P("w2ff", [NB, 128, C], BF16, isOutput=False)
    d["wout"] = P("wout", [NB, 128, C], BF16, isOutput=False)
    d["ffb1"] = P("ffb1", [128, NB], F32, isOutput=False)
    d["outbe"] = P("outbe", [128, NB], F32, isOutput=False)  # wout@ffb2 + outb
    d["cw1"] = P("cw1", [NB, 128, 64], BF16, isOutput=False)
    d["cw2"] = P("cw2", [9, 64, 64], BF16, isOutput=False)
    d["cw3"] = P("cw3", [64, 1], BF16, isOutput=False)
    d["cb1"] = P("cb1", [64, 1], F32, isOutput=False)
    d["cb2"] = P("cb2", [64, 1], F32, isOutput=False)
    d["cb3"] = P("cb3", [1, 1], F32, isOutput=False)
    d["gsel"] = P("gsel", [128, 8], F32, isOutput=False)   # [p, j] = 1/16 if p//16==j
    d["gbct"] = P("gbct", [8, 128], F32, isOutput=False)   # [j, p] = 1 if p//16==j
    d["out"] = P("out", [NB, 128, S], BF16, isOutput=True)

    with tile.TileContext(nc, num_cores=8) as tc:
        _body(nc, tc, d)
    _split_multi_waits(nc)
    return nc


def _gn_coeffs(nc, psp, spool, mv, gamma, beta, gsel_t, gbct_t, eps_t, nm):
    """GroupNorm coefficients from per-(row, block) mean/var stats
    mv [128, NB, 2]: returns per-channel f32 (scale, shift) tiles such that
    y = x*scale + shift."""
    # rq [128, 8]: cols 0..3 = per-row mean (per blk), 4..7 = per-row E[x^2]
    rq = spool.tile([128, 8], F32, tag="gn_rq", name=f"rq{nm}")
    nc.vector.tensor_copy(out=rq[:, 0:NB], in_=mv[:, :, 0])
    nc.vector.scalar_tensor_tensor(
        out=rq[:, NB:2 * NB], in0=mv[:, :, 0], scalar=1.0, in1=mv[:, :, 0],
        op0=ALU.mult, op1=ALU.mult,
    )
    nc.vector.tensor_add(out=rq[:, NB:2 * NB], in0=rq[:, NB:2 * NB], in1=mv[:, :, 1])
    # group-combine (mean over each group's 16 partitions): [8, 8]
    gst_ps = psp.tile([8, 8], F32, tag="sc", name=f"gstps{nm}")
    nc.tensor.matmul(out=gst_ps, lhsT=gsel_t, rhs=rq, start=True, stop=True)
    gst = spool.tile([8, 8], F32, tag="gn_gst", name=f"gst{nm}")
    nc.vector.tensor_copy(out=gst, in_=gst_ps)
    grs = spool.tile([8, NB], F32, tag="gn_grs", name=f"grs{nm}")
    nc.vector.scalar_tensor_tensor(
        out=grs, in0=gst[:, 0:NB], scalar=-1.0, in1=gst[:, 0:NB],
        op0=ALU.mult, op1=ALU.mult,
    )
    nc.vector.tensor_add(out=grs, in0=grs, in1=gst[:, NB:2 * NB])
    nc.scalar.activation(out=grs, in_=grs, func=AF.Sqrt, bias=eps_t[0:8], scale=1.0)
    nc.vector.reciprocal(out=grs, in_=grs)
    # broadcast group values back to channel partitions: [128, 8]
    gmr = spool.tile([8, 8], F32, tag="gn_gmr", name=f"gmr{nm}")
    nc.vector.tensor_copy(out=gmr[:, 0:NB], in_=gst[:, 0:NB])
    nc.vector.tensor_copy(out=gmr[:, NB:2 * NB], in_=grs)
    pcs_ps = psp.tile([128, 8], F32, tag="sc", name=f"pcsps{nm}")
    nc.tensor.matmul(out=pcs_ps, lhsT=gbct_t, rhs=gmr, start=True, stop=True)
    scale = spool.tile([128, NB], F32, tag="gn_scale", name=f"sc{nm}")
    shift = spool.tile([128, NB], F32, tag="gn_shift", name=f"sh{nm}")
    nc.vector.tensor_mul(out=scale, in0=pcs_ps[:, NB:2 * NB], in1=gamma)
    nc.vector.tensor_mul(out=shift, in0=pcs_ps[:, 0:NB], in1=scale)
    nc.vector.tensor_sub(out=shift, in0=beta, in1=shift)
    return scale, shift


def _body(nc, tc, d):
    ctx = contextlib.ExitStack()
    with ctx:
        consts = ctx.enter_context(tc.tile_pool(name="consts", bufs=1))
        gnp = ctx.enter_context(tc.tile_pool(name="gnp", bufs=2))
        psp = ctx.enter_context(tc.tile_pool(name="psp", bufs=2, space="PSUM"))

        # persistent activation pools (created first: stack-order allocation
        # requires the phase-scoped pools below to sit on top)
        shp = ctx.enter_context(tc.tile_pool(name="shp", bufs=1))
        sh_t = shp.tile([128, NB, S], BF16, tag="sh")
        msh = shp.tile([128, NB, S], BF16, tag="msh")
        qkvp = ctx.enter_context(tc.tile_pool(name="qkvp", bufs=1))
        q_t = qkvp.tile([128, NB, S], BF16, tag="q")
        k_t = qkvp.tile([128, NB, S], BF16, tag="k")
        # V interleaved with an all-ones block: [.., h, 0:64]=V_h, [.., 64:128]=1
        vt2 = qkvp.tile([128, NK, NH, 128], BF16, tag="vt")
        nc.any.memset(vt2.rearrange("p k h d -> p (k h) d")[:, :, 64:128], 1.0)
        qm_t = qkvp.tile([128, 2, 2, S], BF16, tag="qm")   # [p, br, mchunk, S]
        km_t = qkvp.tile([128, 2, 2, S], BF16, tag="km")
        vmt_t = qkvp.tile([128, 2, NK, CH], BF16, tag="vmt")
        combp = ctx.enter_context(tc.tile_pool(name="combp", bufs=1))
        comb = combp.tile([128, 2 * NB, S], BF16, tag="comb")

        # ---- input DMAs (x first so GN1 stats can start ASAP) --------------
        ph12 = contextlib.ExitStack()  # pools freed once x/hs/pos are dead
        xp = ph12.enter_context(tc.tile_pool(name="xp", bufs=1))
        # bf16 copy of x for GN1 stats + apply (halves the startup-critical
        # DMA; full-precision x is re-fetched later for the residual)
        x_t = xp.tile([128, NB, S], BF16, tag="x")
        for b in range(NB):
            nc.sync.dma_start(out=x_t[:, b, :], in_=d["xh"].ap()[b])

        posp = ph12.enter_context(tc.tile_pool(name="posp", bufs=1))
        pos_t = posp.tile([128, NB, S], BF16, tag="pos")
        for b in range(NB):
            nc.sync.dma_start(out=pos_t[:, b, :], in_=d["pos"].ap()[b])

        def const_tile(name, shape, dtype):
            t = consts.tile(shape, dtype, tag=name, name=name)
            nc.sync.dma_start(out=t, in_=d[name].ap())
            return t

        def const_blocks(name, nb, w, dtype=BF16):
            t = consts.tile([128, nb, w], dtype, tag=name, name=name)
            nc.sync.dma_start(out=t, in_=d[name].ap().rearrange("b p w -> p b w"))
            return t

        # small / early-needed constants
        gsel_t = const_tile("gsel", [128, 8], F32)
        gbct_t = const_tile("gbct", [8, 128], F32)
        gn_par = {}
        for n in ("pre_g", "pre_b", "norm_g", "norm_b", "post_g", "post_b"):
            gn_par[n] = const_tile(n, [128, NB], F32)
        cw1 = const_blocks("cw1", NB, 64)
        cw2 = consts.tile([64, 9, 64], BF16, tag="cw2", name="cw2")
        nc.sync.dma_start(out=cw2, in_=d["cw2"].ap().rearrange("t p w -> p t w"))
        cw3 = const_tile("cw3", [64, 1], BF16)
        cb1 = const_tile("cb1", [64, 1], F32)
        cb2 = const_tile("cb2", [64, 1], F32)
        cb3 = const_tile("cb3", [1, 1], F32)
        # std projection weights (needed right after GN2)
        wqt = const_blocks("wqt", NB, C)
        wkt = const_blocks("wkt", NB, C)
        wvt = const_blocks("wvt", NB, C)
        # late weights
        wmt = consts.tile([128, 6, NB, CH], BF16, tag="wmt", name="wmt")
        nc.sync.dma_start(out=wmt, in_=d["wmt"].ap().rearrange("i b p w -> p i b w"))
        w1ff = const_blocks("w1ff", 2 * NB, C)
        w2ff = const_blocks("w2ff", NB, C)
        wout = const_blocks("wout", NB, C)
        ffb1 = const_tile("ffb1", [128, NB], F32)
        outbe = const_tile("outbe", [128, NB], F32)

        eps_t = consts.tile([128, 1], F32, tag="eps", name="eps")
        nc.vector.memset(eps_t, EPS)
        ones_mat = consts.tile([128, 128], BF16, tag="ones_mat", name="ones_mat")
        nc.vector.memset(ones_mat, 1.0)
        ones_row = consts.tile([1, 128], BF16, tag="ones_row", name="ones_row")
        nc.vector.memset(ones_row, 1.0)
        zw64 = consts.tile([128, 64], BF16, tag="zw64", name="zw64")
        nc.vector.memset(zw64, 0.0)

        # ---- phase 1: GN1 + pos -> hs ; GN2 -> sh --------------------------
        ph1 = contextlib.ExitStack()
        hsp = ph12.enter_context(tc.tile_pool(name="hsp", bufs=1))
        hs = hsp.tile([128, NB, S], BF16, tag="hs")
        with ph1:
            tmpp = ph1.enter_context(tc.tile_pool(name="tmpp", bufs=2))
            mv1 = gnp.tile([128, NB, 2], F32, tag="gn_mv", name="mv1")
            for b in range(NB):
                st = gnp.tile([128, 2, 6], F32, tag="gn_st", name=f"st1_{b}")
                for h in range(2):
                    nc.vector.bn_stats(out=st[:, h, :],
                                       in_=x_t[:, b, 512 * h:512 * (h + 1)])
                nc.vector.bn_aggr(out=mv1[:, b, :], in_=st)
            sc1, sh1 = _gn_coeffs(nc, psp, gnp, mv1, gn_par["pre_g"],
                                  gn_par["pre_b"], gsel_t, gbct_t, eps_t, "1")
            mv2 = gnp.tile([128, NB, 2], F32, tag="gn_mv", name="mv2")
            for b in range(NB):
                tmp = tmpp.tile([128, S], BF16, tag="hs_tmp", name=f"hst{b}")
                nc.scalar.activation(out=tmp, in_=x_t[:, b, :], func=AF.Identity,
                                     bias=sh1[:, b:b + 1], scale=sc1[:, b:b + 1])
                nc.vector.tensor_add(out=hs[:, b, :], in0=tmp, in1=pos_t[:, b, :])
                st = gnp.tile([128, 2, 6], F32, tag="gn_st", name=f"st2_{b}")
                for h in range(2):
                    nc.vector.bn_stats(out=st[:, h, :],
                                       in_=hs[:, b, 512 * h:512 * (h + 1)])
                nc.vector.bn_aggr(out=mv2[:, b, :], in_=st)
            sc2, sh2 = _gn_coeffs(nc, psp, gnp, mv2, gn_par["norm_g"],
                                  gn_par["norm_b"], gsel_t, gbct_t, eps_t, "2")
            for b in range(NB):
                nc.vector.tensor_scalar(
                    out=sh_t[:, b, :], in0=hs[:, b, :],
                    scalar1=sc2[:, b:b + 1], scalar2=sh2[:, b:b + 1],
                    op0=ALU.mult, op1=ALU.add,
                )

        # ---- phase 2+3: conv spatial gate interleaved with q/k projections -
        # (conv's silu-gated stages would stall the in-order tensor queue, so
        # independent projection chunks are slotted between them)
        def qk_chunk(w_t, dst, m, nm):
            pst = psp.tile([128, S], F32, tag="sc", name=f"proj{nm}{m}")
            for b in range(NB):
                for n in range(2):
                    nc.tensor.matmul(
                        out=pst[:, 512 * n:512 * (n + 1)],
                        lhsT=w_t[:, b, 128 * m:128 * (m + 1)],
                        rhs=sh_t[:, b, 512 * n:512 * (n + 1)],
                        start=(b == 0), stop=(b == NB - 1),
                    )
            nc.any.tensor_copy(out=dst[:, m, :], in_=pst)

        ph2 = contextlib.ExitStack()
        with ph2:
            convp = ph2.enter_context(tc.tile_pool(name="convp", bufs=1))
            PW = 34
            # conv1 1x1 -> silu -> a1 (bf16, zero-padded 34x34)
            c1ps = psp.tile([64, S], F32, tag="pv", name="c1ps")
            for b in range(NB):
                for n in range(2):
                    nc.tensor.matmul(
                        out=c1ps[:, 512 * n:512 * (n + 1)],
                        lhsT=cw1[:, b, :], rhs=hs[:, b, 512 * n:512 * (n + 1)],
                        start=(b == 0), stop=(b == NB - 1),
                    )
            a1b = convp.tile([64, PW * PW], BF16, tag="a1b")
            nc.vector.memset(a1b, 0.0)
            a1bv = a1b.rearrange("p (h w) -> p h w", h=PW)
            c1v = c1ps.rearrange("p (h w) -> p h w", h=HW)
            # cb1 rewritten through a GN2-scale-dependent no-op: orders this
            # silu after GN2's Sqrt in the scalar queue (avoids reloading the
            # sqrt table set between the two) without touching the tensor
            # stream
            cb1b = convp.tile([64, 1], F32, tag="cb1b")
            nc.vector.tensor_scalar(out=cb1b, in0=sc2[0:64, 0:1],
                                    scalar1=0.0, scalar2=None, op0=ALU.mult)
            nc.vector.tensor_add(out=cb1b, in0=cb1b, in1=cb1)
            nc.scalar.activation(out=a1bv[:, 1:33, 1:33], in_=c1v, func=AF.Silu,
                                 bias=cb1b, scale=1.0)
            qk_chunk(wqt, q_t, 0, "q")
            # conv2 3x3 -> silu -> a2 (bf16, padded)
            a2b = convp.tile([64, PW * PW], BF16, tag="a2b")
            nc.vector.memset(a2b, 0.0)
            a2bv = a2b.rearrange("p (h w) -> p h w", h=PW)
            for half in range(2):
                c2ps = psp.tile([64, 512], F32, tag="pv", name=f"c2ps{half}")
                for tap in range(9):
                    ky, kx = tap // 3, tap % 3
                    rv = a1bv[:, 16 * half + ky:16 * half + ky + 16, kx:kx + 32]
                    nc.tensor.matmul(
                        out=c2ps, lhsT=cw2[:, tap, :], rhs=rv,
                        start=(tap == 0), stop=(tap == 8),
                    )
                c2v = c2ps.rearrange("p (h w) -> p h w", h=16)
                nc.scalar.activation(
                    out=a2bv[:, 1 + 16 * half:17 + 16 * half, 1:33],
                    in_=c2v, func=AF.Silu, bias=cb2, scale=1.0,
                )
                qk_chunk(wkt if half else wqt, k_t if half else q_t, 1, "ck")
            # conv3 1x1 -> sigmoid -> sw [1, S]
            c3ps = psp.tile([1, S], F32, tag="pv", name="c3ps")
            for n in range(2):
                rv = a2bv[:, 1 + 16 * n:17 + 16 * n, 1:33]
                nc.tensor.matmul(out=c3ps[:, 512 * n:512 * (n + 1)], lhsT=cw3,
                                 rhs=rv, start=True, stop=True)
            sw = convp.tile([1, S], BF16, tag="sw")
            nc.scalar.activation(out=sw, in_=c3ps, func=AF.Sigmoid, bias=cb3,
                                 scale=1.0)
            qk_chunk(wqt, q_t, 2, "q")
            # broadcast over channels; msh = hs * sigmoid(...)
            swb_ps = psp.tile([128, S], F32, tag="pv", name="swbps")
            for n in range(2):
                nc.tensor.matmul(out=swb_ps[:, 512 * n:512 * (n + 1)],
                                 lhsT=ones_row, rhs=sw[:, 512 * n:512 * (n + 1)],
                                 start=True, stop=True)
            swb = convp.tile([128, S], BF16, tag="swb")
            nc.vector.tensor_copy(out=swb, in_=swb_ps)
            for b in range(NB):
                nc.any.tensor_mul(out=msh[:, b, :], in0=hs[:, b, :], in1=swb)
        ph12.close()  # hs / pos dead; free their SBUF for attention pools

        # ---- remaining projections ----------------------------------------
        inv_sqrt_hd = float(1.0 / np.sqrt(HD))
        inv_sqrt_ch = float(1.0 / np.sqrt(CH))
        qk_chunk(wqt, q_t, 3, "q")
        for m in (0, 2, 3):
            qk_chunk(wkt, k_t, m, "k")


        ph4 = contextlib.ExitStack()
        with ph4:
            ptp = ph4.enter_context(tc.tile_pool(name="ptp", bufs=3))
            rrp = ph4.enter_context(tc.tile_pool(name="rrp", bufs=3))
            pvsp = ph4.enter_context(tc.tile_pool(name="pvsp", bufs=2))

            def sc_exp(pts, ch, kc, hh):
                scp = psp.tile([128, S], F32, tag="sc", name=f"sc{ch}_{kc}_{hh}")
                for n in range(2):
                    nc.tensor.matmul(
                        out=scp[:, 512 * n:512 * (n + 1)],
                        lhsT=k_t[64 * hh:64 * (hh + 1), ch, 128 * kc:128 * (kc + 1)],
                        rhs=q_t[64 * hh:64 * (hh + 1), ch, 512 * n:512 * (n + 1)],
                        start=True, stop=True,
                        tile_position=(64 * hh, 0),
                    )
                nc.scalar.activation(out=pts[hh][:, kc, :], in_=scp,
                                     func=AF.Exp, scale=inv_sqrt_hd)

            def vt2_chunk(m):
                # V^T chunk [128 tokens, C], scattered per-head into vt2
                pst = psp.tile([128, C], F32, tag="sc", name=f"projv{m}")
                for b in range(NB):
                    nc.tensor.matmul(
                        out=pst, lhsT=sh_t[:, b, 128 * m:128 * (m + 1)],
                        rhs=wvt[:, b, :], start=(b == 0), stop=(b == NB - 1),
                    )
                nc.any.tensor_copy(
                    out=vt2[:, m, :, 0:64],
                    in_=pst.rearrange("p (h v) -> p h v", h=NH),
                )

            def ms_proj_group(i):
                # i=0: qm/km br0; i=1: qm/km br1; i=2: vmt br0; i=3: vmt br1
                if i < 2:
                    br = i
                    for (wi, src, dst) in ((0, msh, qm_t), (1, sh_t, km_t)):
                        for m in range(2):
                            pst = psp.tile([128, S], F32, tag="sc",
                                           name=f"pm{br}{wi}{m}")
                            for b in range(NB):
                                for n in range(2):
                                    nc.tensor.matmul(
                                        out=pst[:, 512 * n:512 * (n + 1)],
                                        lhsT=wmt[:, 3 * br + wi, b,
                                                 128 * m:128 * (m + 1)],
                                        rhs=src[:, b, 512 * n:512 * (n + 1)],
                                        start=(b == 0), stop=(b == NB - 1),
                                    )
                            nc.scalar.copy(out=dst[:, br, m, :], in_=pst)
                else:
                    br = i - 2
                    for m in range(NK):
                        pst = psp.tile([128, CH], F32, tag="sc",
                                       name=f"pvm{br}{m}")
                        for b in range(NB):
                            nc.tensor.matmul(
                                out=pst,
                                lhsT=sh_t[:, b, 128 * m:128 * (m + 1)],
                                rhs=wmt[:, 3 * br + 2, b, :],
                                start=(b == 0), stop=(b == NB - 1),
                            )
                        nc.scalar.copy(out=vmt_t[:, br, m, :], in_=pst)

            def ms_sc_exp(ptm, br, kc):
                scp = psp.tile([128, S], F32, tag="sc", name=f"msc{br}_{kc}")
                for n in range(2):
                    for b2 in range(2):
                        nc.tensor.matmul(
                            out=scp[:, 512 * n:512 * (n + 1)],
                            lhsT=km_t[:, br, b2, 128 * kc:128 * (kc + 1)],
                            rhs=qm_t[:, br, b2, 512 * n:512 * (n + 1)],
                            start=(b2 == 0), stop=(b2 == 1),
                        )
                nc.scalar.activation(out=ptm[:, kc, :], in_=scp,
                                     func=AF.Exp, scale=inv_sqrt_ch)

            # pair 0 scores interleaved with V-chunk projections
            pts_cur = [ptp.tile([128, NK, S], BF16, tag="pt", name=f"pt0_{i}")
                       for i in range(2)]
            for kc in range(NK):
                sc_exp(pts_cur, 0, kc, 0)
                sc_exp(pts_cur, 0, kc, 1)
                vt2_chunk(kc)

            # pipelined pairs: PV of pair i + scores of pair i+1 (the multi-
            # scale branch-0 scores play "pair 4" inside pair 3's loop).
            ptm0 = None
            for i in range(NB):
                nxt = None
                if i + 1 < NB:
                    nxt = [ptp.tile([128, NK, S], BF16, tag="pt",
                                    name=f"pt{i + 1}_{j}") for j in range(2)]
                else:
                    ptm0 = ptp.tile([128, NK, S], BF16, tag="pt", name="ptm0")
                pvp = [None, None]
                for hh in range(2):
                    pvp[hh] = psp.tile([128, S], F32, tag="pv",
                                       name=f"pv{i}_{hh}")
                for kc in range(NK):
                    if nxt is not None:
                        sc_exp(nxt, i + 1, kc, 0)
                        sc_exp(nxt, i + 1, kc, 1)
                    else:
                        ms_sc_exp(ptm0, 0, kc)
                    for hh in range(2):
                        h = 2 * i + hh
                        for n in range(2):
                            nc.tensor.matmul(
                                out=pvp[hh][:, 512 * n:512 * (n + 1)],
                                lhsT=vt2[:, kc, h, :],
                                rhs=pts_cur[hh][:, kc, 512 * n:512 * (n + 1)],
                                start=(kc == 0), stop=(kc == NK - 1),
                            )
                # drain PV+denominator out of PSUM fast (frees the banks for
                # the next pair) BEFORE any reciprocal hits the vector queue
                dens, pvsbs = [], []
                for hh in range(2):
                    den = pvsp.tile([64, S], F32, tag="den", name=f"den{i}{hh}")
                    pvsb = pvsp.tile([64, S], BF16, tag="pvsb",
                                     name=f"pvsb{i}{hh}")
                    nc.vector.tensor_copy(out=den, in_=pvp[hh][64:128, :])
                    nc.vector.tensor_copy(out=pvsb, in_=pvp[hh][0:64, :])
                    dens.append(den)
                    pvsbs.append(pvsb)
                for hh in range(2):
                    rr = rrp.tile([64, S], F32, tag="rr", name=f"rr{i}{hh}")
                    nc.vector.reciprocal(out=rr, in_=dens[hh])
                    nc.any.tensor_mul(
                        out=comb[64 * hh:64 * (hh + 1), i, :],
                        in0=pvsbs[hh], in1=rr)
                ms_proj_group(i)
                pts_cur = nxt

            # pass A: branch-1 scores interleaved with branch-0 denominator
            ptm1 = ptp.tile([128, NK, S], BF16, tag="pt", name="ptm1")
            smp0 = psp.tile([128, S], F32, tag="pv", name="smp0")
            for kc in range(NK):
                ms_sc_exp(ptm1, 1, kc)
                for n in range(2):
                    nc.tensor.matmul(
                        out=smp0[:, 512 * n:512 * (n + 1)], lhsT=ones_mat,
                        rhs=ptm0[:, kc, 512 * n:512 * (n + 1)],
                        start=(kc == 0), stop=(kc == NK - 1),
                    )
            densm0 = pvsp.tile([128, S], F32, tag="den", name="densm0")
            nc.vector.tensor_copy(out=densm0, in_=smp0)
            rrm0 = rrp.tile([128, S], F32, tag="rr", name="rrm0")
            nc.vector.reciprocal(out=rrm0, in_=densm0)

            # pass B: branch-0 PV
            pvm0 = [psp.tile([128, S], F32, tag="pv", name=f"pvm0_{m}")
                    for m in range(2)]
            for kc in range(NK):
                for m in range(2):
                    for n in range(2):
                        nc.tensor.matmul(
                            out=pvm0[m][:, 512 * n:512 * (n + 1)],
                            lhsT=vmt_t[:, 0, kc, 128 * m:128 * (m + 1)],
                            rhs=ptm0[:, kc, 512 * n:512 * (n + 1)],
                            start=(kc == 0), stop=(kc == NK - 1),
                        )
            for m in range(2):
                pvsbm = pvsp.tile([128, S], BF16, tag="pvsb",
                                  name=f"pvsbm0{m}")
                nc.vector.tensor_copy(out=pvsbm, in_=pvm0[m])
                nc.any.tensor_mul(out=comb[:, NB + m, :], in0=pvsbm, in1=rrm0)

            # pass C: branch-1 denominator FIRST (its reciprocal then overlaps
            # the branch-1 PV matmuls), then PV
            smp1 = psp.tile([128, S], F32, tag="sc", name="smp1")
            for kc in range(NK):
                for n in range(2):
                    nc.tensor.matmul(
                        out=smp1[:, 512 * n:512 * (n + 1)], lhsT=ones_mat,
                        rhs=ptm1[:, kc, 512 * n:512 * (n + 1)],
                        start=(kc == 0), stop=(kc == NK - 1),
                    )
            densm1 = pvsp.tile([128, S], F32, tag="den", name="densm1")
            nc.vector.tensor_copy(out=densm1, in_=smp1)
            rrm1 = rrp.tile([128, S], F32, tag="rr", name="rrm1")
            nc.vector.reciprocal(out=rrm1, in_=densm1)
            pvm1 = [psp.tile([128, S], F32, tag="pv", name=f"pvm1_{m}")
                    for m in range(2)]
            for kc in range(NK):
                for m in range(2):
                    for n in range(2):
                        nc.tensor.matmul(
                            out=pvm1[m][:, 512 * n:512 * (n + 1)],
                            lhsT=vmt_t[:, 1, kc, 128 * m:128 * (m + 1)],
                            rhs=ptm1[:, kc, 512 * n:512 * (n + 1)],
                            start=(kc == 0), stop=(kc == NK - 1),
                        )
            for m in range(2):
                pvsbm = pvsp.tile([128, S], BF16, tag="pvsb",
                                  name=f"pvsbm1{m}")
                nc.vector.tensor_copy(out=pvsbm, in_=pvm1[m])
                nc.any.tensor_mul(out=comb[:, NB + 2 + m, :], in0=pvsbm,
                                  in1=rrm1)
            # std half of the first FFN accumulations fills the PE while the
            # branch-1 normalize drains on vector
            ff1_pre = [psp.tile([128, S], F32,
                                tag=("sc" if m < 2 else "pv"),
                                name=f"ff1_{m}")
                       for m in range(4)]
            for m in range(4):
                for b in range(NB):
                    for n in range(2):
                        nc.tensor.matmul(
                            out=ff1_pre[m][:, 512 * n:512 * (n + 1)],
                            lhsT=w1ff[:, b, 128 * m:128 * (m + 1)],
                            rhs=comb[:, b, 512 * n:512 * (n + 1)],
                            start=(b == 0), stop=False,
                        )

        # ---- phase 5: FFN + out projection ---------------------------------
        ffp = ctx.enter_context(tc.tile_pool(name="ffp", bufs=1))
        # re-fetch x (bf16) for the residual (x_t's SBUF was recycled after
        # phase 1); bf16 keeps the residual adds in the fast 2x DVE mode
        x2 = ffp.tile([128, NB, S], BF16, tag="x2")
        for b in range(NB):
            nc.sync.dma_start(out=x2[:, b, :], in_=d["xh"].ap()[b])
        act1 = ffp.tile([128, NB, S], BF16, tag="act1")
        for m in range(NB):
            pst = ff1_pre[m]   # std half already accumulated
            for b in range(NB, 2 * NB):
                for n in range(2):
                    nc.tensor.matmul(
                        out=pst[:, 512 * n:512 * (n + 1)],
                        lhsT=w1ff[:, b, 128 * m:128 * (m + 1)],
                        rhs=comb[:, b, 512 * n:512 * (n + 1)],
                        start=False, stop=(b == 2 * NB - 1),
                    )
            nc.scalar.activation(out=act1[:, m, :], in_=pst, func=AF.Silu,
                                 bias=ffb1[:, m:m + 1], scale=1.0)
        f2o = ffp.tile([128, NB, S], BF16, tag="f2o")
        for m in range(NB):
            pst = psp.tile([128, S], F32, tag=("pv" if m % 2 else "sc"), name=f"ff2_{m}")
            for b in range(NB):
                for n in range(2):
                    nc.tensor.matmul(
                        out=pst[:, 512 * n:512 * (n + 1)],
                        lhsT=w2ff[:, b, 128 * m:128 * (m + 1)],
                        rhs=act1[:, b, 512 * n:512 * (n + 1)],
                        start=(b == 0), stop=(b == NB - 1),
                    )
            nc.any.tensor_copy(out=f2o[:, m, :], in_=pst)
        final = ffp.tile([128, NB, S], F32, tag="final")
        mv3 = gnp.tile([128, NB, 2], F32, tag="gn_mv", name="mv3")
        for m in range(NB):
            pst = psp.tile([128, S], F32, tag=("pv" if m % 2 else "sc"), name=f"ffo_{m}")
            for b in range(NB):
                for n in range(2):
                    nc.tensor.matmul(
                        out=pst[:, 512 * n:512 * (n + 1)],
                        lhsT=wout[:, b, 128 * m:128 * (m + 1)],
                        rhs=f2o[:, b, 512 * n:512 * (n + 1)],
                        start=(b == 0), stop=(b == NB - 1),
                    )
            nc.scalar.activation(out=final[:, m, :], in_=pst, func=AF.Identity,
                                 bias=outbe[:, m:m + 1], scale=1.0)
            # GN3 stats straight off the PSUM, in parallel with the bias-apply
            # above; variance is shift-invariant, the mean gets the outbe
            # shift added afterwards
            st = gnp.tile([128, 2, 6], F32, tag="gn_st", name=f"st3_{m}")
            for h in range(2):
                nc.vector.bn_stats(out=st[:, h, :],
                                   in_=pst[:, 512 * h:512 * (h + 1)])
            nc.vector.bn_aggr(out=mv3[:, m, :], in_=st)
            nc.vector.tensor_add(out=mv3[:, m, 0:1], in0=mv3[:, m, 0:1],
                                 in1=outbe[:, m:m + 1])

        # ---- phase 6: GN3 + residual -> out --------------------------------
        sc3, sh3 = _gn_coeffs(nc, psp, gnp, mv3, gn_par["post_g"],
                              gn_par["post_b"], gsel_t, gbct_t, eps_t, "3")
        outt = ffp.tile([128, NB, S], BF16, tag="outt")
        for b in range(NB):
            # alternate engines so the four block-tails pipeline two-wide
            if b % 2 == 0:
                nc.scalar.activation(out=outt[:, b, :], in_=final[:, b, :],
                                     func=AF.Identity, bias=sh3[:, b:b + 1],
                                     scale=sc3[:, b:b + 1])
            else:
                nc.vector.tensor_scalar(
                    out=outt[:, b, :], in0=final[:, b, :],
                    scalar1=sc3[:, b:b + 1], scalar2=sh3[:, b:b + 1],
                    op0=ALU.mult, op1=ALU.add,
                )
            # all-bf16 adds are cheap (2x DVE mode) — keep them on vector;
            # gpsimd's ~2.6 cyc/elem measured 2.5-3.1us here and gated the tail
            nc.vector.tensor_add(out=outt[:, b, :], in0=outt[:, b, :],
                                 in1=x2[:, b, :])
            nc.sync.dma_start(out=d["out"].ap()[b], in_=outt[:, b, :])


# ---------------------------------------------------------------------------
# Host wrapper
# ---------------------------------------------------------------------------
def _prep_inputs(inputs):
    import ml_dtypes
    bf = ml_dtypes.bfloat16
    f8 = ml_dtypes.float8_e4m3
    f32 = np.float32

    def t(a):  # [out, in] -> [in, out]
        return np.ascontiguousarray(np.asarray(a, f32).T)

    def blocks(a, nb):  # [in, out] -> [nb, 128, out]
        return np.ascontiguousarray(np.asarray(a).reshape(nb, 128, -1))

    def chan(a):  # [512] -> [128, 4], channel c = 128*blk + p
        return np.ascontiguousarray(np.asarray(a, f32).reshape(NB, 128).T)

    common = {}
    common["pos"] = np.asarray(inputs["pos_emb"], f32).reshape(C, S).reshape(
        NB, 128, S).astype(bf)
    for n in ("pre_g", "pre_b", "norm_g", "norm_b", "post_g", "post_b"):
        common[n] = chan(inputs[n])
    common["wqt"] = blocks(t(inputs["wq"]), NB).astype(bf)
    common["wkt"] = blocks(t(inputs["wk"]), NB).astype(bf)
    common["wvt"] = blocks(t(inputs["wv"]), NB).astype(bf)
    wm = [t(inputs[k]) for k in ("wq0", "wk0", "wv0", "wq1", "wk1", "wv1")]
    common["wmt"] = np.stack([blocks(w, NB) for w in wm]).astype(bf)
    common["w1ff"] = blocks(t(inputs["ff_w1"]), 2 * NB).astype(bf)
    common["w2ff"] = blocks(t(inputs["ff_w2"]), NB).astype(bf)
    common["wout"] = blocks(t(inputs["out_w"]), NB).astype(bf)
    common["ffb1"] = chan(inputs["ff_b1"])
    out_b_eff = (np.asarray(inputs["out_w"], f32) @ np.asarray(inputs["ff_b2"], f32)
                 + np.asarray(inputs["out_b"], f32))
    common["outbe"] = chan(out_b_eff)
    common["cw1"] = blocks(t(np.asarray(inputs["sa_w1"], f32)[:, :, 0, 0]), NB)\
        .astype(bf)
    w2 = np.asarray(inputs["sa_w2"], f32)  # (64, 64, 3, 3) OIHW
    common["cw2"] = np.stack(
        [np.ascontiguousarray(w2[:, :, ky, kx].T)
         for ky in range(3) for kx in range(3)]).astype(bf)
    common["cw3"] = t(np.asarray(inputs["sa_w3"], f32)[:, :, 0, 0]).astype(bf)
    common["cb1"] = np.asarray(inputs["sa_b1"], f32).reshape(64, 1)
    common["cb2"] = np.asarray(inputs["sa_b2"], f32).reshape(64, 1)
    common["cb3"] = np.asarray(inputs["sa_b3"], f32).reshape(1, 1)
    gsel = np.zeros((128, 8), f32)
    for p in range(128):
        gsel[p, p // GS] = 1.0 / GS
    common["gsel"] = gsel
    gbct = np.zeros((8, 128), f32)
    for p in range(128):
        gbct[p // GS, p] = 1.0
    common["gbct"] = gbct

    xs = np.asarray(inputs["hidden_states"], f32).reshape(B, C, S)
    in_maps = []
    for b in range(B):
        m = dict(common)
        m["x"] = np.ascontiguousarray(xs[b].reshape(NB, 128, S))
        m["xh"] = m["x"].astype(bf)
        in_maps.append(m)
    return in_maps


def kernel(**inputs):
    from concourse.bass_utils import run_bass_kernel_spmd

    nc = bass.Bass("TRN2", debug=False, num_devices=8)
    build(nc)
    in_maps = _prep_inputs(inputs)
    res = run_bass_kernel_spmd(nc, in_maps, core_ids=list(range(8)))
    out = np.stack([np.asarray(res.results[i]["out"], np.float32)
                    for i in range(B)])
    return out.reshape(B, C, HW, HW)


if __name__ == "__main__":
    import reference
    inp = {k: np.asarray(v) for k, v in reference.setup_inputs().items()}
    exp = np.asarray(reference.reference(**inp))
    got = kernel(**inp)
    rel = float(np.linalg.norm(got - exp) / np.linalg.norm(exp))
    print("max abs err:", float(np.abs(got - exp).max()), "rel:", rel)
